# revision 21
# baseline (speedup 1.0000x reference)
"""Causal self-attention (q/k-swapped variant) Bass kernel for Trainium2.

Problem: B=2, T=2048, C=768, H=12, hs=64.
    k = x@Wk+bk ; q = x@Wq+bq ; v = x@Wv+bv          (per-head split)
    att[b,h,i,j] = (k[b,i,h,:] . q[b,j,h,:]) / 8     (note: k rows, q cols)
    att = softmax(causal-mask(att), axis=j)
    y = (att @ v) @ Wo + bo

Sharding: 8 cores = 2 batches x 4 head-groups (3 heads each).
Each core computes its 3 heads fully (QKV proj -> attention -> partial
output projection); host sums the 4 partial outputs per batch and adds bo.

All on-device score math is done in "transposed score" space: score tiles
have j (softmax axis) on partitions and i on the free dim, so the PV matmul
needs no transposes at all, and the softmax denominator falls out of the PV
matmul via an appended ones-column on V.
"""

import os
import sys

sys.path.insert(0, "/opt/trn_rl_repo")

import numpy as np

T = 2048
C = 768
HS = 64
HPC = 3          # heads per core
NCH = C // 128   # 6 contraction chunks
TB = T // 128    # 16 row blocks
JB = T // 128    # 16 j blocks
NCORES = 8
MM_DTYPE = os.environ.get("KERNEL_MM_DTYPE", "fp16")  # fp16 | bf16 | fp32

_cache = {}


def _segments(lo, hi):
    """Split [lo, hi) at 512 boundaries (PSUM bank / fp32 matmul N limit)."""
    out = []
    s = lo
    while s < hi:
        e = min((s // 512 + 1) * 512, hi)
        out.append((s, e))
        s = e
    return out


def _emit(ctx, tc):
    import concourse.bass as bass
    import concourse.tile as tile  # noqa: F401
    from concourse import mybir
    from concourse.bass import ts
    from concourse.masks import make_upper_triangular

    f32 = mybir.dt.float32
    mmd = {"fp16": mybir.dt.float16, "bf16": mybir.dt.bfloat16,
           "fp32": f32}[MM_DTYPE]  # matmul-input dtype
    nc = tc.nc

    xT = nc.dram_tensor("xT", (C, T), mmd, kind="ExternalInput").ap()
    wqk = nc.dram_tensor("wqk", (128, 3 * NCH * 128), mmd, kind="ExternalInput").ap()
    wv = nc.dram_tensor("wv", (128, NCH * 192), mmd, kind="ExternalInput").ap()
    wo = nc.dram_tensor("wo", (64, 3 * C), mmd, kind="ExternalInput").ap()
    bqk = nc.dram_tensor("bqk", (128, 3), f32, kind="ExternalInput").ap()
    bv = nc.dram_tensor("bv", (1, 192), f32, kind="ExternalInput").ap()
    y = nc.dram_tensor("y", (C, T), f32, kind="ExternalOutput").ap()  # transposed

    consts = ctx.enter_context(tc.tile_pool(name="consts", bufs=1))

    # ---- load inputs (wqk + xT first: they gate the first projections) ----
    wqk_sb = consts.tile([128, 3, NCH, 128], mmd)
    nc.sync.dma_start(wqk_sb[:], wqk.rearrange("p (g k m) -> p g k m", g=3, k=NCH))
    xT_sb = consts.tile([128, NCH, T], mmd)
    for k in range(NCH):
        nc.sync.dma_start(xT_sb[:, k, :], xT[k * 128:(k + 1) * 128, :])
    wv_sb = consts.tile([128, NCH, 192], mmd)
    nc.sync.dma_start(wv_sb[:], wv.rearrange("p (k m) -> p k m", k=NCH))
    wo_sb = consts.tile([64, 3, C], mmd)
    nc.sync.dma_start(wo_sb[:], wo.rearrange("p (h c) -> p h c", h=3))
    bqk_sb = consts.tile([128, 3], f32)       # per-partition bias per QK group
    nc.sync.dma_start(bqk_sb[:], bqk)
    bvb_sb = consts.tile([128, 192], f32)     # bv broadcast across partitions
    nc.sync.dma_start(bvb_sb[:], bv.to_broadcast((128, 192)))

    scratch = consts.tile([128, 512], mmd)
    nc.vector.memset(scratch[:], 0.0)
    ones64 = consts.tile([1, 64], f32)
    nc.vector.memset(ones64[:], 1.0)
    trimask = consts.tile([128, 128], mmd)
    make_upper_triangular(nc, trimask[:], val=1.0, diag=True)

    V_aug = consts.tile([128, TB, HPC * 65], mmd)
    for h in range(HPC):
        nc.vector.memset(V_aug[:, :, h * 65 + 64:h * 65 + 65], 1.0)

    QK_sb = consts.tile([128, 3, T], mmd)     # g0=Q(h0,h1) g1=K(h0,h1) g2=[Q(h2)|K(h2)]
    KT2_sb = consts.tile([64, T], mmd)        # K(h2) shifted to base partition 0
    AT_sb = consts.tile([64, HPC, T], mmd)    # normalized attn output, transposed

    # ---- single fused pipeline ----
    # PSUM: psP (proj/outproj, 2 banks) + psS (scores, 4) + psO (Onum, 2) = 8
    psP = ctx.enter_context(tc.tile_pool(name="psP", bufs=2, space="PSUM"))
    psS = ctx.enter_context(tc.tile_pool(name="psS", bufs=2, space="PSUM"))
    psO = ctx.enter_context(tc.tile_pool(name="psO", bufs=1, space="PSUM"))
    sbE = ctx.enter_context(tc.tile_pool(name="E", bufs=4))
    sbATn = ctx.enter_context(tc.tile_pool(name="ATn", bufs=2))
    sbRZ = ctx.enter_context(tc.tile_pool(name="RZ", bufs=2))
    sbY = ctx.enter_context(tc.tile_pool(name="Y", bufs=3))

    # PE warm-up (keeps HAM at full clock while inputs stream in) + exp
    # table pre-load
    for _ in range(28):
        warm = psP.tile([128, 512], f32, tag="p")
        nc.tensor.matmul(warm[:], lhsT=scratch[:, 0:128], rhs=scratch[:],
                         start=True, stop=True, skip_group_check=True)
    edum = sbE.tile([128, 1024], mmd)
    nc.scalar.activation(edum[:, 0:512], scratch[:],
                         mybir.ActivationFunctionType.Exp, scale=0.125)

    def qk_group(g, it):
        ps = psP.tile([128, 512], f32, tag="p")
        for k in range(NCH):
            nc.tensor.matmul(ps[:], lhsT=wqk_sb[:, g, k, :],
                             rhs=xT_sb[:, k, ts(it, 512)],
                             start=(k == 0), stop=(k == NCH - 1))
        nc.vector.tensor_add(QK_sb[:, g, ts(it, 512)], ps[:],
                             bqk_sb[:, g:g + 1].to_broadcast((128, 512)))

    def v_group(tb):
        ps = psP.tile([128, 512], f32, tag="p")
        for k in range(NCH):
            nc.tensor.matmul(ps[:, 0:192], lhsT=xT_sb[:, k, ts(tb, 128)],
                             rhs=wv_sb[:, k, :],
                             start=(k == 0), stop=(k == NCH - 1))
        for h in range(HPC):
            nc.any.tensor_add(V_aug[:, tb, h * 65:h * 65 + 64],
                              ps[:, h * 64:(h + 1) * 64],
                              bvb_sb[:, h * 64:(h + 1) * 64])

    def oproj_group(cb, tt):
        ps = psP.tile([128, 512], f32, tag="p")
        for hh in range(HPC):
            nc.tensor.matmul(ps[:], lhsT=wo_sb[:, hh, ts(cb, 128)],
                             rhs=AT_sb[:, hh, ts(tt, 512)],
                             start=(hh == 0), stop=(hh == HPC - 1))
        ysb = sbY.tile([128, 512], f32)
        nc.any.tensor_copy(ysb[:], ps[:])
        nc.sync.dma_start(y[cb * 128:(cb + 1) * 128, tt * 512:(tt + 1) * 512],
                          ysb[:])

    def kt2_shift():
        nc.sync.dma_start(KT2_sb[:], QK_sb[64:128, 2, :])

    # pre-phase: K^T of h0/h1 (needed in full), Q^T cols for jb 0-3, 3 V
    # blocks; everything else weaves into the chunk stream as PE filler.
    # ST(jb) needs g0 tile jb//4 (filled 1/chunk, 3 chunks ahead); PV(jb)
    # trails by 2 chunks and needs V(jb) (emitted by chunk jb-1).
    for it in range(4):
        qk_group(1, it)
    qk_group(0, 0)
    for tb in range(3):
        v_group(tb)

    from collections import deque
    # pre_fillers carry forward-data hazards (Tile deps are emission-order
    # based!): they MUST all be emitted before the h2 unit that reads
    # g2/KT2. op_fillers (output projection) only read already-emitted data.
    pre_fillers = deque(
        [lambda: qk_group(0, 1), lambda: v_group(3), lambda: qk_group(0, 2),
         lambda: v_group(4), lambda: qk_group(0, 3)]
        + [(lambda tb=tb: v_group(tb)) for tb in range(5, TB)]
        + [(lambda it=it: qk_group(2, it)) for it in range(4)]
        + [kt2_shift]
    )
    op_fillers = deque()

    def pop_filler():
        if pre_fillers:
            pre_fillers.popleft()()
            return True
        if op_fillers:
            op_fillers.popleft()()
            return True
        return False

    # per-head (lhsT=Q^T, rhs=K^T) access patterns; partition bases match
    heads = [
        (QK_sb[0:64, 0, :], QK_sb[0:64, 1, :]),
        (QK_sb[64:128, 0, :], QK_sb[64:128, 1, :]),
        (QK_sb[0:64, 2, :], KT2_sb[:, :]),
    ]

    HW = 1024  # i-window per (half, head) unit
    for half in range(T // HW):
        c0 = HW * half
        njb = (c0 + HW) // 128
        for h in range(HPC):
            if h == 2:
                while pre_fillers:  # h2 reads g2/KT2: emit their writers now
                    pre_fillers.popleft()()
            QT, KT = heads[h]
            Onum = psO.tile([65, HW], f32)

            def emit_pv(jb, E, lo):
                for a, b in _segments(lo, c0 + HW):
                    nc.tensor.matmul(Onum[:, a - c0:b - c0],
                                     lhsT=V_aug[:, jb, h * 65:(h + 1) * 65],
                                     rhs=E[:, a - c0:b - c0],
                                     start=(jb == 0),
                                     stop=(jb == min(4 * (a // 512) + 3,
                                                     njb - 1)),
                                     skip_group_check=True)

            pending = []
            for jb in range(njb):
                i0 = 128 * jb
                lo = max(c0, i0)
                S = psS.tile([128, HW], f32)
                for a, b in _segments(lo, c0 + HW):
                    nc.tensor.matmul(S[:, a - c0:b - c0],
                                     lhsT=QT[:, ts(jb, 128)],
                                     rhs=KT[:, a:b], start=True, stop=True)
                E = sbE.tile([128, HW], mmd)
                nc.scalar.activation(E[:, lo - c0:], S[:, lo - c0:],
                                     mybir.ActivationFunctionType.Exp,
                                     scale=0.125)
                if lo == i0:  # window containing the diagonal block
                    r = i0 - c0
                    nc.vector.tensor_mul(E[:, r:r + 128], E[:, r:r + 128],
                                         trimask[:])
                if pop_filler():
                    pass
                else:
                    # dummy full-array matmul: holds the PE activity monitor
                    # at full clock through ACT-paced attention stretches
                    warm = psP.tile([128, 512], f32, tag="p")
                    nc.tensor.matmul(warm[:], lhsT=scratch[:, 0:128],
                                     rhs=scratch[:], start=True, stop=True,
                                     skip_group_check=True)
                pending.append((jb, E, lo))
                if len(pending) > 2:  # PV trails ST by 2 chunks
                    emit_pv(*pending.pop(0))
            for item in pending:
                emit_pv(*item)

            # prompt copy frees Onum for the next unit; row 64 is Z.
            # Z sits on one partition: DMA-reshape to [128, HW/128] for a
            # parallel reciprocal, fold back, then replicate across 64
            # partitions with rank-1 matmuls and divide.
            ATn = sbATn.tile([65, HW], f32)
            nc.vector.tensor_copy(ATn[:], Onum[:])
            z16 = sbRZ.tile([128, HW // 128], f32, tag="z16")
            nc.sync.dma_start(z16[:], ATn[64:65, :])
            r16 = sbRZ.tile([128, HW // 128], f32, tag="r16")
            nc.vector.reciprocal(r16[:], z16[:])
            rz1 = sbRZ.tile([1, HW], f32, tag="rz1")
            nc.sync.dma_start(rz1[:], r16[:])
            for m in range(HW // 512):
                slot = psP.tile([128, 512], f32, tag="p")
                nc.tensor.matmul(slot[0:64, :], lhsT=ones64[:],
                                 rhs=rz1[0:1, ts(m, 512)], start=True,
                                 stop=True, skip_group_check=True)
                nc.vector.tensor_mul(AT_sb[:, h, c0 + m * 512:c0 + (m + 1) * 512],
                                     ATn[0:64, ts(m, 512)], slot[0:64, :])

        # all heads done for this half: its output columns can project out;
        # groups run as fillers inside the next half (or drain at the end)
        for cb in range(NCH):
            for tt in range(c0 // 512, (c0 + HW) // 512):
                op_fillers.append(lambda cb=cb, tt=tt: oproj_group(cb, tt))

    # drain remaining fillers (the last half's output projection); dummies
    # keep the PE clock up through the normalization-chain latency
    for _ in range(10):
        warm = psP.tile([128, 512], f32, tag="p")
        nc.tensor.matmul(warm[:], lhsT=scratch[:, 0:128], rhs=scratch[:],
                         start=True, stop=True, skip_group_check=True)
    while pre_fillers or op_fillers:
        pop_filler()


def _build():
    if "nc" in _cache:
        return _cache["nc"]
    from contextlib import ExitStack

    import concourse.tile as tile
    from concourse import bacc

    nc = bacc.Bacc("TRN2", target_bir_lowering=False, debug=False,
                   num_devices=NCORES)
    with tile.TileContext(nc) as tc:
        with ExitStack() as ctx:
            _emit(ctx, tc)
    nc.compile()
    _cache["nc"] = nc
    return nc


def _install_trace_hooks():
    """Make trace=True work in this container: shim the missing
    antenv.axon_hooks NTFF-profile hook (ctypes into libaxon_pjrt.so) and
    skip the S3 artifact upload."""
    import contextlib
    import ctypes
    import types

    import concourse.bass_utils as bu

    bu.upload_artifacts = lambda tmpdir: tmpdir
    try:
        from antenv.axon_hooks import get_axon_ntff_profile_hook  # noqa: F401
        return
    except ImportError:
        pass

    so_path = "/opt/axon/libaxon_pjrt.so"
    if not os.path.exists(so_path):
        return
    lib = ctypes.CDLL(so_path)
    if not hasattr(lib, "axon_start_nrt_profile"):
        return
    lib.axon_start_nrt_profile.argtypes = [
        ctypes.POINTER(ctypes.c_int64), ctypes.c_size_t,
    ]
    lib.axon_start_nrt_profile.restype = ctypes.c_int64
    lib.axon_stop_nrt_profile.argtypes = [ctypes.c_char_p]
    lib.axon_stop_nrt_profile.restype = ctypes.c_int64

    @contextlib.contextmanager
    def _hook(output_dir, device_ids):
        import jax
        jax.devices()
        if device_ids:
            ids = (ctypes.c_int64 * len(device_ids))(*device_ids)
            rc = lib.axon_start_nrt_profile(ids, len(device_ids))
        else:
            rc = lib.axon_start_nrt_profile(None, 0)
        if rc != 0:
            raise RuntimeError(f"axon_start_nrt_profile rc={rc}")
        try:
            yield
        finally:
            n = lib.axon_stop_nrt_profile(str(output_dir).encode())
            print(f"profile: {n} file(s) written to {output_dir}",
                  file=sys.stderr)

    state = {"h": _hook}
    mod = types.ModuleType("antenv.axon_hooks")
    mod.get_axon_ntff_profile_hook = lambda: state["h"]
    mod.set_axon_ntff_profile_hook = lambda h: state.__setitem__("h", h)
    import antenv
    antenv.axon_hooks = mod
    sys.modules["antenv.axon_hooks"] = mod


def kernel(**inputs):
    x = np.ascontiguousarray(np.asarray(inputs["x"], dtype=np.float32))
    Wq = np.asarray(inputs["Wq"], dtype=np.float32)
    Wk = np.asarray(inputs["Wk"], dtype=np.float32)
    Wv = np.asarray(inputs["Wv"], dtype=np.float32)
    Wo = np.asarray(inputs["Wo"], dtype=np.float32)
    bq = np.asarray(inputs["bq"], dtype=np.float32)
    bk = np.asarray(inputs["bk"], dtype=np.float32)
    bv = np.asarray(inputs["bv"], dtype=np.float32)
    bo = np.asarray(inputs["bo"], dtype=np.float32)

    from concourse import bass_utils

    nc = _build()

    if MM_DTYPE == "bf16":
        import ml_dtypes
        mmd_np = ml_dtypes.bfloat16
    elif MM_DTYPE == "fp16":
        mmd_np = np.float16
    else:
        mmd_np = np.float32

    B = x.shape[0]
    xTs = [np.ascontiguousarray(x[b].T.astype(mmd_np)) for b in range(B)]
    in_maps = []
    for core in range(NCORES):
        b, hg = core // 4, core % 4
        sl = slice(hg * 192, (hg + 1) * 192)
        wq_s, wk_s = Wq[:, sl], Wk[:, sl]
        g0 = wq_s[:, 0:128]
        g1 = wk_s[:, 0:128]
        g2 = np.concatenate([wq_s[:, 128:192], wk_s[:, 128:192]], axis=1)
        wqk_h = (np.stack([g0, g1, g2], 0)
                 .reshape(3, NCH, 128, 128).transpose(2, 0, 1, 3)
                 .reshape(128, 3 * NCH * 128))
        wv_h = (Wv[:, sl].reshape(NCH, 128, 192).transpose(1, 0, 2)
                .reshape(128, NCH * 192))
        wo_h = (Wo[sl, :].reshape(3, 64, C).transpose(1, 0, 2)
                .reshape(64, 3 * C))
        bqk_h = np.stack(
            [bq[sl][0:128], bk[sl][0:128],
             np.concatenate([bq[sl][128:192], bk[sl][128:192]])], axis=1
        )  # [128, 3]
        bv_h = bv[sl].reshape(1, 192)
        in_maps.append({
            "xT": xTs[b],
            "wqk": np.ascontiguousarray(wqk_h.astype(mmd_np)),
            "wv": np.ascontiguousarray(wv_h.astype(mmd_np)),
            "wo": np.ascontiguousarray(wo_h.astype(mmd_np)),
            "bqk": np.ascontiguousarray(bqk_h),
            "bv": np.ascontiguousarray(bv_h),
        })

    trace = bool(os.environ.get("KERNEL_TRACE"))
    if trace:
        _install_trace_hooks()
    res = bass_utils.run_bass_kernel_spmd(
        nc, in_maps, core_ids=list(range(NCORES)), trace=trace
    )
    _cache["last_results"] = res

    out = np.empty((B, T, C), dtype=np.float32)
    for b in range(B):
        acc = res.results[b * 4]["y"].copy()
        for hg in range(1, 4):
            acc += res.results[b * 4 + hg]["y"]
        out[b] = acc.T + bo
    return out


# revision 22
# speedup vs baseline: 1.0686x; 1.0686x over previous
"""Causal self-attention (q/k-swapped variant) Bass kernel for Trainium2.

Problem: B=2, T=2048, C=768, H=12, hs=64.
    k = x@Wk+bk ; q = x@Wq+bq ; v = x@Wv+bv          (per-head split)
    att[b,h,i,j] = (k[b,i,h,:] . q[b,j,h,:]) / 8     (note: k rows, q cols)
    att = softmax(causal-mask(att), axis=j)
    y = (att @ v) @ Wo + bo

Sharding: 8 cores = 2 batches x 4 head-groups (3 heads each).
Each core computes its 3 heads fully (QKV proj -> attention -> partial
output projection); host sums the 4 partial outputs per batch and adds bo.

All on-device score math is done in "transposed score" space: score tiles
have j (softmax axis) on partitions and i on the free dim, so the PV matmul
needs no transposes at all, and the softmax denominator falls out of the PV
matmul via an appended ones-column on V.
"""

import os
import sys

sys.path.insert(0, "/opt/trn_rl_repo")

import numpy as np

T = 2048
C = 768
HS = 64
HPC = 3          # heads per core
NCH = C // 128   # 6 contraction chunks
TB = T // 128    # 16 row blocks
JB = T // 128    # 16 j blocks
NCORES = 8
MM_DTYPE = os.environ.get("KERNEL_MM_DTYPE", "fp16")  # fp16 | bf16 | fp32

_cache = {}


def _segments(lo, hi):
    """Split [lo, hi) at 512 boundaries (PSUM bank / fp32 matmul N limit)."""
    out = []
    s = lo
    while s < hi:
        e = min((s // 512 + 1) * 512, hi)
        out.append((s, e))
        s = e
    return out


def _emit(ctx, tc):
    import concourse.bass as bass
    import concourse.tile as tile  # noqa: F401
    from concourse import mybir
    from concourse.bass import ts
    from concourse.masks import make_upper_triangular

    f32 = mybir.dt.float32
    mmd = {"fp16": mybir.dt.float16, "bf16": mybir.dt.bfloat16,
           "fp32": f32}[MM_DTYPE]  # matmul-input dtype
    nc = tc.nc

    xT = nc.dram_tensor("xT", (C, T), mmd, kind="ExternalInput").ap()
    wqk = nc.dram_tensor("wqk", (128, 3 * NCH * 128), mmd, kind="ExternalInput").ap()
    wv = nc.dram_tensor("wv", (128, NCH * 192), mmd, kind="ExternalInput").ap()
    wo = nc.dram_tensor("wo", (64, 3 * C), mmd, kind="ExternalInput").ap()
    bqk = nc.dram_tensor("bqk", (128, 3), f32, kind="ExternalInput").ap()
    bv = nc.dram_tensor("bv", (1, 192), f32, kind="ExternalInput").ap()
    y = nc.dram_tensor("y", (C, T), f32, kind="ExternalOutput").ap()  # transposed

    consts = ctx.enter_context(tc.tile_pool(name="consts", bufs=1))

    # ---- load inputs (wqk + xT first: they gate the first projections) ----
    wqk_sb = consts.tile([128, 3, NCH, 128], mmd)
    nc.sync.dma_start(wqk_sb[:], wqk.rearrange("p (g k m) -> p g k m", g=3, k=NCH))
    xT_sb = consts.tile([128, NCH, T], mmd)
    for k in range(NCH):
        nc.sync.dma_start(xT_sb[:, k, :], xT[k * 128:(k + 1) * 128, :])
    wv_sb = consts.tile([128, NCH, 192], mmd)
    nc.sync.dma_start(wv_sb[:], wv.rearrange("p (k m) -> p k m", k=NCH))
    wo_sb = consts.tile([64, 3, C], mmd)
    nc.sync.dma_start(wo_sb[:], wo.rearrange("p (h c) -> p h c", h=3))
    bqk_sb = consts.tile([128, 3], f32)       # per-partition bias per QK group
    nc.sync.dma_start(bqk_sb[:], bqk)
    bvb_sb = consts.tile([128, 192], f32)     # bv broadcast across partitions
    nc.sync.dma_start(bvb_sb[:], bv.to_broadcast((128, 192)))

    scratch = consts.tile([128, 512], mmd)
    nc.vector.memset(scratch[:], 0.0)
    ones64 = consts.tile([1, 64], f32)
    nc.vector.memset(ones64[:], 1.0)
    trimask = consts.tile([128, 128], mmd)
    make_upper_triangular(nc, trimask[:], val=1.0, diag=True)

    V_aug = consts.tile([128, TB, HPC * 65], mmd)
    for h in range(HPC):
        nc.vector.memset(V_aug[:, :, h * 65 + 64:h * 65 + 65], 1.0)

    QK_sb = consts.tile([128, 3, T], mmd)     # g0=Q(h0,h1) g1=K(h0,h1) g2=[Q(h2)|K(h2)]
    KT2_sb = consts.tile([64, T], mmd)        # K(h2) shifted to base partition 0
    AT_sb = consts.tile([64, HPC, T], mmd)    # normalized attn output, transposed

    # ---- single fused pipeline ----
    # PSUM: psP (proj/outproj, 2 banks) + psS (scores, 4) + psO (Onum, 2) = 8
    psP = ctx.enter_context(tc.tile_pool(name="psP", bufs=2, space="PSUM"))
    psS = ctx.enter_context(tc.tile_pool(name="psS", bufs=2, space="PSUM"))
    psO = ctx.enter_context(tc.tile_pool(name="psO", bufs=1, space="PSUM"))
    sbE = ctx.enter_context(tc.tile_pool(name="E", bufs=4))
    sbATn = ctx.enter_context(tc.tile_pool(name="ATn", bufs=2))
    sbRZ = ctx.enter_context(tc.tile_pool(name="RZ", bufs=2))
    sbY = ctx.enter_context(tc.tile_pool(name="Y", bufs=3))

    # PE warm-up (keeps HAM at full clock while inputs stream in) + exp
    # table pre-load
    for _ in range(28):
        warm = psP.tile([128, 512], f32, tag="p")
        nc.tensor.matmul(warm[:], lhsT=scratch[:, 0:128], rhs=scratch[:],
                         start=True, stop=True, skip_group_check=True)
    edum = sbE.tile([128, 1024], mmd)
    nc.scalar.activation(edum[:, 0:512], scratch[:],
                         mybir.ActivationFunctionType.Exp, scale=0.125)

    def qk_group(g, it):
        ps = psP.tile([128, 512], f32, tag="p")
        for k in range(NCH):
            nc.tensor.matmul(ps[:], lhsT=wqk_sb[:, g, k, :],
                             rhs=xT_sb[:, k, ts(it, 512)],
                             start=(k == 0), stop=(k == NCH - 1))
        nc.vector.tensor_add(QK_sb[:, g, ts(it, 512)], ps[:],
                             bqk_sb[:, g:g + 1].to_broadcast((128, 512)))

    def v_group(tb):
        ps = psP.tile([128, 512], f32, tag="p")
        for k in range(NCH):
            nc.tensor.matmul(ps[:, 0:192], lhsT=xT_sb[:, k, ts(tb, 128)],
                             rhs=wv_sb[:, k, :],
                             start=(k == 0), stop=(k == NCH - 1))
        for h in range(HPC):
            nc.any.tensor_add(V_aug[:, tb, h * 65:h * 65 + 64],
                              ps[:, h * 64:(h + 1) * 64],
                              bvb_sb[:, h * 64:(h + 1) * 64])

    def oproj_group(cb, tt):
        ps = psP.tile([128, 512], f32, tag="p")
        for hh in range(HPC):
            nc.tensor.matmul(ps[:], lhsT=wo_sb[:, hh, ts(cb, 128)],
                             rhs=AT_sb[:, hh, ts(tt, 512)],
                             start=(hh == 0), stop=(hh == HPC - 1))
        ysb = sbY.tile([128, 512], f32)
        nc.any.tensor_copy(ysb[:], ps[:])
        nc.sync.dma_start(y[cb * 128:(cb + 1) * 128, tt * 512:(tt + 1) * 512],
                          ysb[:])

    def kt2_shift():
        nc.sync.dma_start(KT2_sb[:], QK_sb[64:128, 2, :])

    # pre-phase: K^T of h0/h1 (needed in full), Q^T cols for jb 0-3, 3 V
    # blocks; everything else weaves into the chunk stream as PE filler.
    # ST(jb) needs g0 tile jb//4 (filled 1/chunk, 3 chunks ahead); PV(jb)
    # trails by 2 chunks and needs V(jb) (emitted by chunk jb-1).
    for it in range(4):
        qk_group(1, it)
    qk_group(0, 0)
    for tb in range(3):
        v_group(tb)

    from collections import deque
    # pre_fillers carry forward-data hazards (Tile deps are emission-order
    # based!): they MUST all be emitted before the h2 unit that reads
    # g2/KT2. op_fillers (output projection) only read already-emitted data.
    pre_fillers = deque(
        [lambda: qk_group(0, 1), lambda: v_group(3), lambda: qk_group(0, 2),
         lambda: v_group(4), lambda: qk_group(0, 3)]
        + [(lambda tb=tb: v_group(tb)) for tb in range(5, TB)]
        + [(lambda it=it: qk_group(2, it)) for it in range(4)]
        + [kt2_shift]
    )
    op_fillers = deque()

    def pop_filler():
        if pre_fillers:
            pre_fillers.popleft()()
            return True
        if op_fillers:
            op_fillers.popleft()()
            return True
        return False

    # per-head (lhsT=Q^T, rhs=K^T) access patterns; partition bases match
    heads = [
        (QK_sb[0:64, 0, :], QK_sb[0:64, 1, :]),
        (QK_sb[64:128, 0, :], QK_sb[64:128, 1, :]),
        (QK_sb[0:64, 2, :], KT2_sb[:, :]),
    ]

    HW = 1024  # i-window per (half, head) unit
    for half in range(T // HW):
        c0 = HW * half
        njb = (c0 + HW) // 128
        for h in range(HPC):
            if h == 2:
                while pre_fillers:  # h2 reads g2/KT2: emit their writers now
                    pre_fillers.popleft()()
            QT, KT = heads[h]
            Onum = psO.tile([65, HW], f32)

            def emit_pv(jb, E, lo):
                for a, b in _segments(lo, c0 + HW):
                    nc.tensor.matmul(Onum[:, a - c0:b - c0],
                                     lhsT=V_aug[:, jb, h * 65:(h + 1) * 65],
                                     rhs=E[:, a - c0:b - c0],
                                     start=(jb == 0),
                                     stop=(jb == min(4 * (a // 512) + 3,
                                                     njb - 1)),
                                     skip_group_check=True)

            pending = []
            for jb in range(njb):
                i0 = 128 * jb
                lo = max(c0, i0)
                S = psS.tile([128, HW], f32)
                for a, b in _segments(lo, c0 + HW):
                    nc.tensor.matmul(S[:, a - c0:b - c0],
                                     lhsT=QT[:, ts(jb, 128)],
                                     rhs=KT[:, a:b], start=True, stop=True)
                E = sbE.tile([128, HW], mmd)
                nc.scalar.activation(E[:, lo - c0:], S[:, lo - c0:],
                                     mybir.ActivationFunctionType.Exp,
                                     scale=0.125)
                if lo == i0:  # window containing the diagonal block
                    r = i0 - c0
                    nc.vector.tensor_mul(E[:, r:r + 128], E[:, r:r + 128],
                                         trimask[:])
                if pop_filler():
                    pass
                else:
                    # dummy full-array matmul: holds the PE activity monitor
                    # at full clock through ACT-paced attention stretches
                    warm = psP.tile([128, 512], f32, tag="p")
                    nc.tensor.matmul(warm[:], lhsT=scratch[:, 0:128],
                                     rhs=scratch[:], start=True, stop=True,
                                     skip_group_check=True)
                pending.append((jb, E, lo))
                if len(pending) > 2:  # PV trails ST by 2 chunks
                    emit_pv(*pending.pop(0))
            for item in pending:
                emit_pv(*item)

            # prompt copy frees Onum for the next unit; row 64 is Z.
            # Z sits on one partition: DMA-reshape to [128, HW/128] for a
            # parallel reciprocal, fold back, then replicate across 64
            # partitions with rank-1 matmuls and divide.
            ATn = sbATn.tile([65, HW], f32)
            nc.vector.tensor_copy(ATn[:], Onum[:])
            z16 = sbRZ.tile([128, HW // 128], f32, tag="z16")
            nc.sync.dma_start(z16[:], ATn[64:65, :])
            r16 = sbRZ.tile([128, HW // 128], f32, tag="r16")
            nc.vector.reciprocal(r16[:], z16[:])
            rz1 = sbRZ.tile([1, HW], f32, tag="rz1")
            nc.sync.dma_start(rz1[:], r16[:])
            rzb = sbRZ.tile([64, HW], f32, tag="rzb")
            nc.gpsimd.partition_broadcast(rzb[:], rz1[:], channels=64)
            nc.vector.tensor_mul(AT_sb[:, h, c0:c0 + HW], ATn[0:64, :], rzb[:])

        # all heads done for this half: its output columns can project out;
        # groups run as fillers inside the next half (or drain at the end)
        for cb in range(NCH):
            for tt in range(c0 // 512, (c0 + HW) // 512):
                op_fillers.append(lambda cb=cb, tt=tt: oproj_group(cb, tt))

    # drain remaining fillers (the last half's output projection); dummies
    # keep the PE clock up through the normalization-chain latency
    for _ in range(10):
        warm = psP.tile([128, 512], f32, tag="p")
        nc.tensor.matmul(warm[:], lhsT=scratch[:, 0:128], rhs=scratch[:],
                         start=True, stop=True, skip_group_check=True)
    while pre_fillers or op_fillers:
        pop_filler()
        warm = psP.tile([128, 512], f32, tag="p")
        nc.tensor.matmul(warm[:], lhsT=scratch[:, 0:128], rhs=scratch[:],
                         start=True, stop=True, skip_group_check=True)


def _build():
    if "nc" in _cache:
        return _cache["nc"]
    from contextlib import ExitStack

    import concourse.tile as tile
    from concourse import bacc

    nc = bacc.Bacc("TRN2", target_bir_lowering=False, debug=False,
                   num_devices=NCORES)
    with tile.TileContext(nc) as tc:
        with ExitStack() as ctx:
            _emit(ctx, tc)
    nc.compile()
    _cache["nc"] = nc
    return nc


def _install_trace_hooks():
    """Make trace=True work in this container: shim the missing
    antenv.axon_hooks NTFF-profile hook (ctypes into libaxon_pjrt.so) and
    skip the S3 artifact upload."""
    import contextlib
    import ctypes
    import types

    import concourse.bass_utils as bu

    bu.upload_artifacts = lambda tmpdir: tmpdir
    try:
        from antenv.axon_hooks import get_axon_ntff_profile_hook  # noqa: F401
        return
    except ImportError:
        pass

    so_path = "/opt/axon/libaxon_pjrt.so"
    if not os.path.exists(so_path):
        return
    lib = ctypes.CDLL(so_path)
    if not hasattr(lib, "axon_start_nrt_profile"):
        return
    lib.axon_start_nrt_profile.argtypes = [
        ctypes.POINTER(ctypes.c_int64), ctypes.c_size_t,
    ]
    lib.axon_start_nrt_profile.restype = ctypes.c_int64
    lib.axon_stop_nrt_profile.argtypes = [ctypes.c_char_p]
    lib.axon_stop_nrt_profile.restype = ctypes.c_int64

    @contextlib.contextmanager
    def _hook(output_dir, device_ids):
        import jax
        jax.devices()
        if device_ids:
            ids = (ctypes.c_int64 * len(device_ids))(*device_ids)
            rc = lib.axon_start_nrt_profile(ids, len(device_ids))
        else:
            rc = lib.axon_start_nrt_profile(None, 0)
        if rc != 0:
            raise RuntimeError(f"axon_start_nrt_profile rc={rc}")
        try:
            yield
        finally:
            n = lib.axon_stop_nrt_profile(str(output_dir).encode())
            print(f"profile: {n} file(s) written to {output_dir}",
                  file=sys.stderr)

    state = {"h": _hook}
    mod = types.ModuleType("antenv.axon_hooks")
    mod.get_axon_ntff_profile_hook = lambda: state["h"]
    mod.set_axon_ntff_profile_hook = lambda h: state.__setitem__("h", h)
    import antenv
    antenv.axon_hooks = mod
    sys.modules["antenv.axon_hooks"] = mod


def kernel(**inputs):
    x = np.ascontiguousarray(np.asarray(inputs["x"], dtype=np.float32))
    Wq = np.asarray(inputs["Wq"], dtype=np.float32)
    Wk = np.asarray(inputs["Wk"], dtype=np.float32)
    Wv = np.asarray(inputs["Wv"], dtype=np.float32)
    Wo = np.asarray(inputs["Wo"], dtype=np.float32)
    bq = np.asarray(inputs["bq"], dtype=np.float32)
    bk = np.asarray(inputs["bk"], dtype=np.float32)
    bv = np.asarray(inputs["bv"], dtype=np.float32)
    bo = np.asarray(inputs["bo"], dtype=np.float32)

    from concourse import bass_utils

    nc = _build()

    if MM_DTYPE == "bf16":
        import ml_dtypes
        mmd_np = ml_dtypes.bfloat16
    elif MM_DTYPE == "fp16":
        mmd_np = np.float16
    else:
        mmd_np = np.float32

    B = x.shape[0]
    xTs = [np.ascontiguousarray(x[b].T.astype(mmd_np)) for b in range(B)]
    in_maps = []
    for core in range(NCORES):
        b, hg = core // 4, core % 4
        sl = slice(hg * 192, (hg + 1) * 192)
        wq_s, wk_s = Wq[:, sl], Wk[:, sl]
        g0 = wq_s[:, 0:128]
        g1 = wk_s[:, 0:128]
        g2 = np.concatenate([wq_s[:, 128:192], wk_s[:, 128:192]], axis=1)
        wqk_h = (np.stack([g0, g1, g2], 0)
                 .reshape(3, NCH, 128, 128).transpose(2, 0, 1, 3)
                 .reshape(128, 3 * NCH * 128))
        wv_h = (Wv[:, sl].reshape(NCH, 128, 192).transpose(1, 0, 2)
                .reshape(128, NCH * 192))
        wo_h = (Wo[sl, :].reshape(3, 64, C).transpose(1, 0, 2)
                .reshape(64, 3 * C))
        bqk_h = np.stack(
            [bq[sl][0:128], bk[sl][0:128],
             np.concatenate([bq[sl][128:192], bk[sl][128:192]])], axis=1
        )  # [128, 3]
        bv_h = bv[sl].reshape(1, 192)
        in_maps.append({
            "xT": xTs[b],
            "wqk": np.ascontiguousarray(wqk_h.astype(mmd_np)),
            "wv": np.ascontiguousarray(wv_h.astype(mmd_np)),
            "wo": np.ascontiguousarray(wo_h.astype(mmd_np)),
            "bqk": np.ascontiguousarray(bqk_h),
            "bv": np.ascontiguousarray(bv_h),
        })

    trace = bool(os.environ.get("KERNEL_TRACE"))
    if trace:
        _install_trace_hooks()
    res = bass_utils.run_bass_kernel_spmd(
        nc, in_maps, core_ids=list(range(NCORES)), trace=trace
    )
    _cache["last_results"] = res

    out = np.empty((B, T, C), dtype=np.float32)
    for b in range(B):
        acc = res.results[b * 4]["y"].copy()
        for hg in range(1, 4):
            acc += res.results[b * 4 + hg]["y"]
        out[b] = acc.T + bo
    return out


# revision 23
# speedup vs baseline: 1.1172x; 1.0455x over previous
"""Causal self-attention (q/k-swapped variant) Bass kernel for Trainium2.

Problem: B=2, T=2048, C=768, H=12, hs=64.
    k = x@Wk+bk ; q = x@Wq+bq ; v = x@Wv+bv          (per-head split)
    att[b,h,i,j] = (k[b,i,h,:] . q[b,j,h,:]) / 8     (note: k rows, q cols)
    att = softmax(causal-mask(att), axis=j)
    y = (att @ v) @ Wo + bo

Sharding: 8 cores = 2 batches x 4 head-groups (3 heads each).
Each core computes its 3 heads fully (QKV proj -> attention -> partial
output projection); host sums the 4 partial outputs per batch and adds bo.

All on-device score math is done in "transposed score" space: score tiles
have j (softmax axis) on partitions and i on the free dim, so the PV matmul
needs no transposes at all, and the softmax denominator falls out of the PV
matmul via an appended ones-column on V.
"""

import os
import sys

sys.path.insert(0, "/opt/trn_rl_repo")

import numpy as np

T = 2048
C = 768
HS = 64
HPC = 3          # heads per core
NCH = C // 128   # 6 contraction chunks
TB = T // 128    # 16 row blocks
JB = T // 128    # 16 j blocks
NCORES = 8
MM_DTYPE = os.environ.get("KERNEL_MM_DTYPE", "fp16")  # fp16 | bf16 | fp32

_cache = {}


def _segments(lo, hi):
    """Split [lo, hi) at 512 boundaries (PSUM bank / fp32 matmul N limit)."""
    out = []
    s = lo
    while s < hi:
        e = min((s // 512 + 1) * 512, hi)
        out.append((s, e))
        s = e
    return out


def _emit(ctx, tc):
    import concourse.bass as bass
    import concourse.tile as tile  # noqa: F401
    from concourse import mybir
    from concourse.bass import ts
    from concourse.masks import make_upper_triangular

    f32 = mybir.dt.float32
    mmd = {"fp16": mybir.dt.float16, "bf16": mybir.dt.bfloat16,
           "fp32": f32}[MM_DTYPE]  # matmul-input dtype
    nc = tc.nc

    xT = nc.dram_tensor("xT", (C, T), mmd, kind="ExternalInput").ap()
    wqk = nc.dram_tensor("wqk", (128, 3 * NCH * 128), mmd, kind="ExternalInput").ap()
    wv = nc.dram_tensor("wv", (128, NCH * 192), mmd, kind="ExternalInput").ap()
    wo = nc.dram_tensor("wo", (64, 3 * C), mmd, kind="ExternalInput").ap()
    bqk = nc.dram_tensor("bqk", (128, 3), f32, kind="ExternalInput").ap()
    bv = nc.dram_tensor("bv", (1, 192), f32, kind="ExternalInput").ap()
    y = nc.dram_tensor("y", (C, T), f32, kind="ExternalOutput").ap()  # transposed

    consts = ctx.enter_context(tc.tile_pool(name="consts", bufs=1))

    # ---- load inputs (wqk + xT first: they gate the first projections) ----
    wqk_sb = consts.tile([128, 3, NCH, 128], mmd)
    nc.sync.dma_start(wqk_sb[:], wqk.rearrange("p (g k m) -> p g k m", g=3, k=NCH))
    xT_sb = consts.tile([128, NCH, T], mmd)
    for k in range(NCH):
        nc.sync.dma_start(xT_sb[:, k, :], xT[k * 128:(k + 1) * 128, :])
    wv_sb = consts.tile([128, NCH, 192], mmd)
    nc.sync.dma_start(wv_sb[:], wv.rearrange("p (k m) -> p k m", k=NCH))
    wo_sb = consts.tile([64, 3, C], mmd)
    nc.sync.dma_start(wo_sb[:], wo.rearrange("p (h c) -> p h c", h=3))
    bqk_sb = consts.tile([128, 3], f32)       # per-partition bias per QK group
    nc.sync.dma_start(bqk_sb[:], bqk)
    bvb_sb = consts.tile([128, 192], f32)     # bv broadcast across partitions
    nc.sync.dma_start(bvb_sb[:], bv.to_broadcast((128, 192)))

    scratch = consts.tile([128, 512], mmd)
    nc.vector.memset(scratch[:], 0.0)
    ones64 = consts.tile([1, 64], f32)
    nc.vector.memset(ones64[:], 1.0)
    trimask = consts.tile([128, 128], mmd)
    make_upper_triangular(nc, trimask[:], val=1.0, diag=True)

    V_aug = consts.tile([128, TB, HPC * 65], mmd)
    for h in range(HPC):
        nc.vector.memset(V_aug[:, :, h * 65 + 64:h * 65 + 65], 1.0)

    QK_sb = consts.tile([128, 3, T], mmd)     # g0=Q(h0,h1) g1=K(h0,h1) g2=[Q(h2)|K(h2)]
    KT2_sb = consts.tile([64, T], mmd)        # K(h2) shifted to base partition 0
    AT_sb = consts.tile([64, HPC, T], mmd)    # normalized attn output, transposed

    # ---- single fused pipeline ----
    # PSUM: psP (proj/outproj, 2 banks) + psS (scores, 4) + psO (Onum, 2) = 8
    psP = ctx.enter_context(tc.tile_pool(name="psP", bufs=2, space="PSUM"))
    psS = ctx.enter_context(tc.tile_pool(name="psS", bufs=2, space="PSUM"))
    psO = ctx.enter_context(tc.tile_pool(name="psO", bufs=1, space="PSUM"))
    sbE = ctx.enter_context(tc.tile_pool(name="E", bufs=4))
    sbATn = ctx.enter_context(tc.tile_pool(name="ATn", bufs=2))
    sbRZ = ctx.enter_context(tc.tile_pool(name="RZ", bufs=2))
    sbY = ctx.enter_context(tc.tile_pool(name="Y", bufs=3))

    # PE warm-up (keeps HAM at full clock while inputs stream in) + exp
    # table pre-load
    for _ in range(28):
        warm = psP.tile([128, 512], f32, tag="p")
        nc.tensor.matmul(warm[:], lhsT=scratch[:, 0:128], rhs=scratch[:],
                         start=True, stop=True, skip_group_check=True)
    edum = sbE.tile([128, 1024], mmd)
    nc.scalar.activation(edum[:, 0:512], scratch[:],
                         mybir.ActivationFunctionType.Exp, scale=0.125)

    def qk_group(g, it):
        ps = psP.tile([128, 512], f32, tag="p")
        for k in range(NCH):
            nc.tensor.matmul(ps[:], lhsT=wqk_sb[:, g, k, :],
                             rhs=xT_sb[:, k, ts(it, 512)],
                             start=(k == 0), stop=(k == NCH - 1))
        nc.vector.tensor_add(QK_sb[:, g, ts(it, 512)], ps[:],
                             bqk_sb[:, g:g + 1].to_broadcast((128, 512)))

    def v_group(tb):
        ps = psP.tile([128, 512], f32, tag="p")
        for k in range(NCH):
            nc.tensor.matmul(ps[:, 0:192], lhsT=xT_sb[:, k, ts(tb, 128)],
                             rhs=wv_sb[:, k, :],
                             start=(k == 0), stop=(k == NCH - 1))
        for h in range(HPC):
            nc.any.tensor_add(V_aug[:, tb, h * 65:h * 65 + 64],
                              ps[:, h * 64:(h + 1) * 64],
                              bvb_sb[:, h * 64:(h + 1) * 64])

    def oproj_group(cb, tt):
        ps = psP.tile([128, 512], f32, tag="p")
        for hh in range(HPC):
            nc.tensor.matmul(ps[:], lhsT=wo_sb[:, hh, ts(cb, 128)],
                             rhs=AT_sb[:, hh, ts(tt, 512)],
                             start=(hh == 0), stop=(hh == HPC - 1))
        ysb = sbY.tile([128, 512], f32)
        nc.any.tensor_copy(ysb[:], ps[:])
        nc.sync.dma_start(y[cb * 128:(cb + 1) * 128, tt * 512:(tt + 1) * 512],
                          ysb[:])

    def kt2_shift():
        nc.sync.dma_start(KT2_sb[:], QK_sb[64:128, 2, :])

    # pre-phase: K^T of h0/h1 (needed in full), Q^T cols for jb 0-3, 3 V
    # blocks; everything else weaves into the chunk stream as PE filler.
    # ST(jb) needs g0 tile jb//4 (filled 1/chunk, 3 chunks ahead); PV(jb)
    # trails by 2 chunks and needs V(jb) (emitted by chunk jb-1).
    for it in range(4):
        qk_group(1, it)
    qk_group(0, 0)
    for tb in range(3):
        v_group(tb)

    from collections import deque
    # pre_fillers carry forward-data hazards (Tile deps are emission-order
    # based!): they MUST all be emitted before the h2 unit that reads
    # g2/KT2. op_fillers (output projection) only read already-emitted data.
    pre_fillers = deque(
        [lambda: qk_group(0, 1), lambda: v_group(3), lambda: qk_group(0, 2),
         lambda: v_group(4), lambda: qk_group(0, 3)]
        + [(lambda tb=tb: v_group(tb)) for tb in range(5, TB)]
        + [(lambda it=it: qk_group(2, it)) for it in range(4)]
        + [kt2_shift]
    )
    op_fillers = deque()

    def pop_filler():
        if pre_fillers:
            pre_fillers.popleft()()
            return True
        if op_fillers:
            op_fillers.popleft()()
            return True
        return False

    # per-head (lhsT=Q^T, rhs=K^T) access patterns; partition bases match
    heads = [
        (QK_sb[0:64, 0, :], QK_sb[0:64, 1, :]),
        (QK_sb[64:128, 0, :], QK_sb[64:128, 1, :]),
        (QK_sb[0:64, 2, :], KT2_sb[:, :]),
    ]

    HW = 1024  # i-window per (half, head) unit
    for half in range(T // HW):
        c0 = HW * half
        njb = (c0 + HW) // 128
        for h in range(HPC):
            if h == 2:
                while pre_fillers:  # h2 reads g2/KT2: emit their writers now
                    pre_fillers.popleft()()
            QT, KT = heads[h]
            Onum = psO.tile([65, HW], f32)

            def emit_pv(jb, E, lo):
                for a, b in _segments(lo, c0 + HW):
                    nc.tensor.matmul(Onum[:, a - c0:b - c0],
                                     lhsT=V_aug[:, jb, h * 65:(h + 1) * 65],
                                     rhs=E[:, a - c0:b - c0],
                                     start=(jb == 0),
                                     stop=(jb == min(4 * (a // 512) + 3,
                                                     njb - 1)),
                                     skip_group_check=True)

            pending = []
            for jb in range(njb):
                i0 = 128 * jb
                lo = max(c0, i0)
                S = psS.tile([128, HW], f32)
                for a, b in _segments(lo, c0 + HW):
                    nc.tensor.matmul(S[:, a - c0:b - c0],
                                     lhsT=QT[:, ts(jb, 128)],
                                     rhs=KT[:, a:b], start=True, stop=True)
                E = sbE.tile([128, HW], mmd)
                nc.scalar.activation(E[:, lo - c0:], S[:, lo - c0:],
                                     mybir.ActivationFunctionType.Exp,
                                     scale=0.125)
                if lo == i0:  # window containing the diagonal block
                    r = i0 - c0
                    nc.vector.tensor_mul(E[:, r:r + 128], E[:, r:r + 128],
                                         trimask[:])
                if not pop_filler():
                    # dummy full-array matmuls: hold the PE activity monitor
                    # at full clock through ACT-paced attention stretches
                    for _ in range(2):
                        warm = psP.tile([128, 512], f32, tag="p")
                        nc.tensor.matmul(warm[:], lhsT=scratch[:, 0:128],
                                         rhs=scratch[:], start=True, stop=True,
                                         skip_group_check=True)
                pending.append((jb, E, lo))
                if len(pending) > 2:  # PV trails ST by 2 chunks
                    emit_pv(*pending.pop(0))
            for item in pending:
                emit_pv(*item)

            # prompt copy frees Onum for the next unit; row 64 is Z.
            # Z sits on one partition: DMA-reshape to [128, HW/128] for a
            # parallel reciprocal, fold back, then replicate across 64
            # partitions with rank-1 matmuls and divide.
            ATn = sbATn.tile([65, HW], f32)
            nc.vector.tensor_copy(ATn[:], Onum[:])
            z16 = sbRZ.tile([128, HW // 128], f32, tag="z16")
            nc.sync.dma_start(z16[:], ATn[64:65, :])
            r16 = sbRZ.tile([128, HW // 128], f32, tag="r16")
            nc.vector.reciprocal(r16[:], z16[:])
            rz1 = sbRZ.tile([1, HW], f32, tag="rz1")
            nc.sync.dma_start(rz1[:], r16[:])
            rzb = sbRZ.tile([64, HW], f32, tag="rzb")
            nc.gpsimd.partition_broadcast(rzb[:], rz1[:], channels=64)
            nc.vector.tensor_mul(AT_sb[:, h, c0:c0 + HW], ATn[0:64, :], rzb[:])

        # all heads done for this half: its output columns can project out;
        # groups run as fillers inside the next half (or drain at the end)
        for cb in range(NCH):
            for tt in range(c0 // 512, (c0 + HW) // 512):
                op_fillers.append(lambda cb=cb, tt=tt: oproj_group(cb, tt))

    # drain remaining fillers (the last half's output projection); dummies
    # keep the PE clock up through the normalization-chain latency
    for _ in range(10):
        warm = psP.tile([128, 512], f32, tag="p")
        nc.tensor.matmul(warm[:], lhsT=scratch[:, 0:128], rhs=scratch[:],
                         start=True, stop=True, skip_group_check=True)
    while pre_fillers or op_fillers:
        pop_filler()
        warm = psP.tile([128, 512], f32, tag="p")
        nc.tensor.matmul(warm[:], lhsT=scratch[:, 0:128], rhs=scratch[:],
                         start=True, stop=True, skip_group_check=True)


def _build():
    if "nc" in _cache:
        return _cache["nc"]
    from contextlib import ExitStack

    import concourse.tile as tile
    from concourse import bacc

    nc = bacc.Bacc("TRN2", target_bir_lowering=False, debug=False,
                   num_devices=NCORES)
    with tile.TileContext(nc) as tc:
        with ExitStack() as ctx:
            _emit(ctx, tc)
    nc.compile()
    _cache["nc"] = nc
    return nc


def _install_trace_hooks():
    """Make trace=True work in this container: shim the missing
    antenv.axon_hooks NTFF-profile hook (ctypes into libaxon_pjrt.so) and
    skip the S3 artifact upload."""
    import contextlib
    import ctypes
    import types

    import concourse.bass_utils as bu

    bu.upload_artifacts = lambda tmpdir: tmpdir
    try:
        from antenv.axon_hooks import get_axon_ntff_profile_hook  # noqa: F401
        return
    except ImportError:
        pass

    so_path = "/opt/axon/libaxon_pjrt.so"
    if not os.path.exists(so_path):
        return
    lib = ctypes.CDLL(so_path)
    if not hasattr(lib, "axon_start_nrt_profile"):
        return
    lib.axon_start_nrt_profile.argtypes = [
        ctypes.POINTER(ctypes.c_int64), ctypes.c_size_t,
    ]
    lib.axon_start_nrt_profile.restype = ctypes.c_int64
    lib.axon_stop_nrt_profile.argtypes = [ctypes.c_char_p]
    lib.axon_stop_nrt_profile.restype = ctypes.c_int64

    @contextlib.contextmanager
    def _hook(output_dir, device_ids):
        import jax
        jax.devices()
        if device_ids:
            ids = (ctypes.c_int64 * len(device_ids))(*device_ids)
            rc = lib.axon_start_nrt_profile(ids, len(device_ids))
        else:
            rc = lib.axon_start_nrt_profile(None, 0)
        if rc != 0:
            raise RuntimeError(f"axon_start_nrt_profile rc={rc}")
        try:
            yield
        finally:
            n = lib.axon_stop_nrt_profile(str(output_dir).encode())
            print(f"profile: {n} file(s) written to {output_dir}",
                  file=sys.stderr)

    state = {"h": _hook}
    mod = types.ModuleType("antenv.axon_hooks")
    mod.get_axon_ntff_profile_hook = lambda: state["h"]
    mod.set_axon_ntff_profile_hook = lambda h: state.__setitem__("h", h)
    import antenv
    antenv.axon_hooks = mod
    sys.modules["antenv.axon_hooks"] = mod


def kernel(**inputs):
    x = np.ascontiguousarray(np.asarray(inputs["x"], dtype=np.float32))
    Wq = np.asarray(inputs["Wq"], dtype=np.float32)
    Wk = np.asarray(inputs["Wk"], dtype=np.float32)
    Wv = np.asarray(inputs["Wv"], dtype=np.float32)
    Wo = np.asarray(inputs["Wo"], dtype=np.float32)
    bq = np.asarray(inputs["bq"], dtype=np.float32)
    bk = np.asarray(inputs["bk"], dtype=np.float32)
    bv = np.asarray(inputs["bv"], dtype=np.float32)
    bo = np.asarray(inputs["bo"], dtype=np.float32)

    from concourse import bass_utils

    nc = _build()

    if MM_DTYPE == "bf16":
        import ml_dtypes
        mmd_np = ml_dtypes.bfloat16
    elif MM_DTYPE == "fp16":
        mmd_np = np.float16
    else:
        mmd_np = np.float32

    B = x.shape[0]
    xTs = [np.ascontiguousarray(x[b].T.astype(mmd_np)) for b in range(B)]
    in_maps = []
    for core in range(NCORES):
        b, hg = core // 4, core % 4
        sl = slice(hg * 192, (hg + 1) * 192)
        wq_s, wk_s = Wq[:, sl], Wk[:, sl]
        g0 = wq_s[:, 0:128]
        g1 = wk_s[:, 0:128]
        g2 = np.concatenate([wq_s[:, 128:192], wk_s[:, 128:192]], axis=1)
        wqk_h = (np.stack([g0, g1, g2], 0)
                 .reshape(3, NCH, 128, 128).transpose(2, 0, 1, 3)
                 .reshape(128, 3 * NCH * 128))
        wv_h = (Wv[:, sl].reshape(NCH, 128, 192).transpose(1, 0, 2)
                .reshape(128, NCH * 192))
        wo_h = (Wo[sl, :].reshape(3, 64, C).transpose(1, 0, 2)
                .reshape(64, 3 * C))
        bqk_h = np.stack(
            [bq[sl][0:128], bk[sl][0:128],
             np.concatenate([bq[sl][128:192], bk[sl][128:192]])], axis=1
        )  # [128, 3]
        bv_h = bv[sl].reshape(1, 192)
        in_maps.append({
            "xT": xTs[b],
            "wqk": np.ascontiguousarray(wqk_h.astype(mmd_np)),
            "wv": np.ascontiguousarray(wv_h.astype(mmd_np)),
            "wo": np.ascontiguousarray(wo_h.astype(mmd_np)),
            "bqk": np.ascontiguousarray(bqk_h),
            "bv": np.ascontiguousarray(bv_h),
        })

    trace = bool(os.environ.get("KERNEL_TRACE"))
    if trace:
        _install_trace_hooks()
    res = bass_utils.run_bass_kernel_spmd(
        nc, in_maps, core_ids=list(range(NCORES)), trace=trace
    )
    _cache["last_results"] = res

    out = np.empty((B, T, C), dtype=np.float32)
    for b in range(B):
        acc = res.results[b * 4]["y"].copy()
        for hg in range(1, 4):
            acc += res.results[b * 4 + hg]["y"]
        out[b] = acc.T + bo
    return out
